# revision 12
# baseline (speedup 1.0000x reference)
"""Trainium2 Bass kernel for nn_BertClassifier_77309411685 (V14).

Data-parallel over 8 NeuronCores: each core handles 256 samples; the small
base linear and 12 expert heads are replicated.

V14 = the proven V7 pipeline (all means on the PE via diag stationaries,
per-group tails interleaved so group-0's transpose/close/expert chain runs
during group-1's gather) + two measured wins:
  * wbT shipped HOST-PREARRANGED in the [128, KC*INNER] SBUF layout so the
    weight stream is 128 x multi-KB contiguous descriptors.  V7's strided
    weight view trickled on 512-B descriptors until 33us and the teardown
    barrier waited ~4.5us on it.
  * samples per core are permuted by span length (host-side; un-permuted
    on host): group 0 = 128 longest spans (JB rows, gathered first),
    group 1 = 128 shortest (JA~5 rows).  Cuts gather bytes ~20% and the
    PE mean-matmul work by ~2.5us.
Other V7 elements kept verbatim: big single-descriptor gathers (12KB/8KB),
gidx on the gpsimd queue (same-engine completion tracking), diag(mask/len)
stationaries built on Vector, PE warm-up (now memset-fed, no DMA dep),
per-group expert select via is_equal + strided reduce, per-group output
DMAs.
"""

import numpy as np
from contextlib import ExitStack

import concourse.bass as bass
import concourse.tile as tile
from concourse import bacc, mybir
from concourse.bass import IndirectOffsetOnAxis
from concourse.bass_utils import run_bass_kernel_spmd

F32 = mybir.dt.float32
F16 = mybir.dt.float16
I32 = mybir.dt.int32

B, S, H = 2048, 256, 768
INNER, NB_CTX, NB_EXPERTS, NB_LABELS = 256, 2, 12, 3
NCORES = 8
BC = B // NCORES             # 256 samples per core
F3H = (NB_CTX + 1) * H       # 2304
KC = F3H // 128              # 18 contraction chunks
HC = H // 128                # 6 chunks per feature block
NE = NB_EXPERTS * NB_LABELS  # 36
EROWS = BC * S               # rows in the per-core embedding tensor

# The reference picks 2 static context positions host-side with this exact rng.
CTX_IDX = [int(v) for v in np.random.default_rng(seed=0).choice(np.arange(S), size=NB_CTX)]

MUL = mybir.AluOpType.mult


def _build(JA, JB):
    JS = [JB, JA]                                # rows per group (g0=B, g1=A)
    MOFF = NE + 2                                # mask cols offset in c32

    nc = bacc.Bacc(
        "TRN2",
        target_bir_lowering=False,
        debug=False,
        enable_asserts=False,
        num_devices=NCORES,
    )
    embT = nc.dram_tensor("embT", [EROWS, H], F16, kind="ExternalInput").ap()
    gidx = nc.dram_tensor("gidx", [128, 2], I32, kind="ExternalInput").ap()
    # wbT pre-arranged: wbT[p, c*INNER + m] = W_base[m, c*128 + p]
    wbT = nc.dram_tensor("wbT", [128, KC * INNER], F16, kind="ExternalInput").ap()
    ctxT = nc.dram_tensor("ctxT", [128, NB_CTX * HC * 256], F16, kind="ExternalInput").ap()
    # c16: diag(1/len) g0 [0:128) g1 [128:256) + identity [256:384)
    #      + wexpA + wexpB
    c16 = nc.dram_tensor("c16", [128, 3 * 128 + 2 * NE], F16, kind="ExternalInput").ap()
    # c32: io36 [0:36) + categories-as-float [36:38) + span masks (JB+JA)
    #      + b_base (t p) layout (2)
    c32 = nc.dram_tensor("c32", [128, MOFF + JB + JA + 2], F32, kind="ExternalInput").ap()
    # c1: ones row [0:256) + expert bias row [256:292)
    c1 = nc.dram_tensor("c1", [1, 256 + NE], F16, kind="ExternalInput").ap()
    out = nc.dram_tensor("out", [BC, NB_LABELS], F32, kind="ExternalOutput").ap()

    with tile.TileContext(nc) as tc, ExitStack() as ctx:
        pool = ctx.enter_context(tc.tile_pool(name="main", bufs=1))
        pst = ctx.enter_context(tc.tile_pool(name="pst", bufs=1, space="PSUM"))

        # --- phase 0: tiny front-of-queue loads the gathers depend on ---
        gidx_t = pool.tile([128, 2], I32)
        nc.gpsimd.dma_start(gidx_t[:], gidx[:, :])

        # --- phase 1: span gathers - one op per group, JS[g] contiguous
        # rows from `start` as a single descriptor per sample ---
        g_h = []
        for g in range(2):
            gt = pool.tile([128, JS[g] * H], F16, tag=f"g{g}", bufs=1)
            nc.gpsimd.indirect_dma_start(
                out=gt[:], out_offset=None, in_=embT,
                in_offset=IndirectOffsetOnAxis(ap=gidx_t[:, g:g + 1], axis=0),
            )
            g_h.append(gt)

        # --- phase 2: weight streams (contiguous descriptors) ---
        featT = pool.tile([128, KC * 256], F16)
        wbT_t = pool.tile([128, KC * INNER], F16)
        nc.sync.dma_start(wbT_t[:, HC * INNER:], wbT[:, HC * INNER:])
        nc.sync.dma_start(featT[:, HC * 256:KC * 256], ctxT[:, :])
        nc.sync.dma_start(wbT_t[:, :HC * INNER], wbT[:, :HC * INNER])

        c16_t = pool.tile([128, 3 * 128 + 2 * NE], F16)
        nc.scalar.dma_start(c16_t[:], c16[:, :])
        wexpA = c16_t[:, 384:384 + NE]
        wexpB = c16_t[:, 384 + NE:384 + 2 * NE]
        c32_t = pool.tile([128, MOFF + JB + JA + 2], F32)
        nc.scalar.dma_start(c32_t[:], c32[:, :])
        io36f = c32_t[:, 0:NE]
        catf = c32_t[:, NE:NE + 2]
        bb_t = c32_t[:, MOFF + JB + JA:MOFF + JB + JA + 2]
        c1_t = pool.tile([1, 256 + NE], F16)
        nc.scalar.dma_start(c1_t[:], c1[:, :])
        ones1 = c1_t[:, 0:256]
        wexpC = c1_t[:, 256:256 + NE]

        # PE warm-up from a memset tile (no DMA dependency): the HAM clock
        # gate releases after sustained activity.
        warm_src = pool.tile([128, 256], F16)
        nc.vector.memset(warm_src[:], 0.0)
        warm = pst.tile([128, 256], F32, tag="psb", bufs=1)
        for w in range(8):
            nc.tensor.matmul(warm[:], lhsT=warm_src[:, 0:128], rhs=warm_src[:],
                             start=(w == 0), stop=(w == 7))

        # --- phase 3b/3c/4 interleaved per group (V7 structure) ---
        accs_h = [[pst.tile([128, 128], F32, tag=f"acc{g}{mt}", bufs=1,
                            name=f"acc{g}{mt}") for mt in range(2)]
                  for g in range(2)]

        def ctx_mms(g):
            for c in range(HC, KC):
                for mt in range(2):
                    nc.tensor.matmul(
                        accs_h[g][mt][:],
                        lhsT=wbT_t[:, c * INNER + mt * 128: c * INNER + (mt + 1) * 128],
                        rhs=featT[:, c * 256 + g * 128: c * 256 + g * 128 + 128],
                        start=(c == HC), stop=False,
                    )

        # diag(mask_j/len) stationaries, built on Vector from diag(1/len)
        # x 0/1 span masks
        dmask = [pool.tile([128, JS[g] * 128], F16, name=f"dmask{g}")
                 for g in range(2)]
        moff_g = [MOFF, MOFF + JB]
        for g in range(2):
            diag = c16_t[:, g * 128:(g + 1) * 128]
            for j in range(JS[g]):
                nc.vector.tensor_scalar(
                    dmask[g][:, j * 128:(j + 1) * 128], diag,
                    c32_t[:, moff_g[g] + j:moff_g[g] + j + 1], None,
                    op0=MUL)

        featT_pairs = featT[:].rearrange("p (c x) -> p c x", x=256)
        hiddenT = pool.tile([128, 2 * 256], F16)
        identity = c16_t[:, 256:384]
        out3 = pool.tile([128, 2 * NB_LABELS], F32)  # [p, g*3 + n]
        outv = out.rearrange("(g p) n -> p g n", p=128)

        ctx_mms(0)
        ctx_mms(1)

        # masked mean on the PE, both groups back to back so g1's matmuls
        # never queue behind g0's downstream chain
        ps_h = []
        for g in range(2):
            gt = g_h[g]
            psa = pst.tile([128, 512], F32, tag=f"psa{g}", bufs=1)
            psb = pst.tile([128, 256], F32, tag="psb", bufs=1, name=f"psb{g}")
            for j in range(JS[g]):
                dm = dmask[g][:, j * 128:(j + 1) * 128]
                nc.tensor.matmul(psa[:], lhsT=dm, rhs=gt[:, j * H:j * H + 512],
                                 start=(j == 0), stop=(j == JS[g] - 1))
                nc.tensor.matmul(psb[:], lhsT=dm,
                                 rhs=gt[:, j * H + 512:(j + 1) * H],
                                 start=(j == 0), stop=(j == JS[g] - 1))
            ps_h.append((psa, psb))

        for g in range(2):
            psa, psb = ps_h[g]
            ct = pool.tile([128, H], F16, tag=f"ct{g}", bufs=1)
            # drain the mean PSUM split across Scalar and Vector
            nc.scalar.copy(ct[:, 0:512], psa[:])
            nc.vector.tensor_copy(ct[:, 512:768], psb[:])

            # center transposes; all 6 chunks land in one PSUM bank, drained
            # by two strided copies (Scalar + Vector in parallel) into featT
            tpc = pst.tile([128, HC * 128], F16, tag="tpc", bufs=1)
            for c in range(HC):
                nc.tensor.transpose(tpc[:, c * 128:(c + 1) * 128],
                                    ct[:, c * 128:(c + 1) * 128], identity)
            tpcv = tpc[:].rearrange("p (c x) -> p c x", c=HC)
            nc.scalar.copy(
                featT_pairs[:, 0:3, g * 128:(g + 1) * 128], tpcv[:, 0:3, :])
            nc.vector.tensor_copy(
                featT_pairs[:, 3:HC, g * 128:(g + 1) * 128], tpcv[:, 3:HC, :])

            # center chunks close the base-linear accumulation; bias+relu
            for c in range(HC):
                for mt in range(2):
                    nc.tensor.matmul(
                        accs_h[g][mt][:],
                        lhsT=wbT_t[:, c * INNER + mt * 128: c * INNER + (mt + 1) * 128],
                        rhs=featT[:, c * 256 + g * 128: c * 256 + g * 128 + 128],
                        start=False, stop=(c == HC - 1),
                    )
            for mt in range(2):
                nc.scalar.activation(
                    hiddenT[:, mt * 256 + g * 128: mt * 256 + g * 128 + 128],
                    accs_h[g][mt][:],
                    mybir.ActivationFunctionType.Relu,
                    bias=bb_t[:, mt:mt + 1], scale=1.0)

            # expert heads + per-sample selection, inline per group
            b0 = g * 128
            mask36 = pool.tile([128, NE], F32, tag=f"mask36{g}", bufs=1)
            nc.vector.tensor_scalar(mask36[:], io36f, catf[:, g:g + 1], None,
                                    op0=mybir.AluOpType.is_equal)
            ps36 = ps_h[g][0][:, 0:NE]
            nc.tensor.matmul(ps36, lhsT=hiddenT[:, b0:b0 + 128],
                             rhs=wexpA, start=True, stop=False)
            nc.tensor.matmul(ps36, lhsT=hiddenT[:, 256 + b0:256 + b0 + 128],
                             rhs=wexpB, start=False, stop=False)
            nc.tensor.matmul(ps36, lhsT=ones1[0:1, b0:b0 + 128],
                             rhs=wexpC, start=False, stop=True)

            prod = pool.tile([128, NE], F32, tag=f"prod{g}", bufs=1)
            nc.vector.tensor_tensor(out=prod[:], in0=ps36, in1=mask36[:],
                                    op=MUL)
            nc.vector.tensor_reduce(
                out=out3[:, g * NB_LABELS:(g + 1) * NB_LABELS],
                in_=prod[:].rearrange("p (e n) -> p n e", n=NB_LABELS),
                axis=mybir.AxisListType.X, op=mybir.AluOpType.add)
            nc.sync.dma_start(
                outv[:, g:g + 1, :],
                out3[:].rearrange("p (x n) -> p x n", n=NB_LABELS)[:, g:g + 1, :])

    nc.compile()
    return nc


_NC = {}


def _get_nc(JA, JB):
    key = (JA, JB)
    if key not in _NC:
        _NC[key] = _build(JA, JB)
    return _NC[key]


def _prep_inputs(embeddings, position_indexes, categories, W_base, b_base,
                 W_experts, b_experts):
    emb32 = np.asarray(embeddings)
    emb16 = emb32.astype(np.float16).reshape(NCORES, BC, S, H)

    pos = np.asarray(position_indexes).astype(np.int64).reshape(NCORES, BC, 2)
    cat = np.asarray(categories).astype(np.int64).reshape(NCORES, BC)

    lens_all = pos[:, :, 1] - pos[:, :, 0]                     # [NC, 256]
    perm = np.argsort(lens_all, axis=1, kind="stable")
    # group order: g0 = longest 128 (gathered first), g1 = shortest
    permP = np.concatenate([perm[:, 128:], perm[:, :128]], axis=1)
    lensP = np.take_along_axis(lens_all, permP, 1)
    startsP = np.take_along_axis(pos[:, :, 0], permP, 1)
    catP = np.take_along_axis(cat, permP, 1)

    JB = int(lensP[:, :128].max())
    JA = int(lensP[:, 128:].max())
    assert 1 <= JA <= 8 and 1 <= JB <= 8

    row = (permP * S + startsP).astype(np.int32)               # [NC, 256]
    gidx = np.stack([row[:, :128], row[:, 128:]], axis=2)      # [NC, 128, 2]

    # base linear: wbT[p, c*INNER+m] = W_base[m, c*128+p], shipped contiguous
    wb = np.asarray(W_base, dtype=np.float32)
    wbT = np.ascontiguousarray(
        wb.T.reshape(KC, 128, INNER).transpose(1, 0, 2).reshape(128, KC * INNER)
    ).astype(np.float16)

    bbias = np.asarray(b_base, dtype=np.float32)

    we = np.asarray(W_experts, dtype=np.float32)
    be = np.asarray(b_experts, dtype=np.float32)
    wexp = we.transpose(2, 0, 1).reshape(INNER, NE)
    eye = np.eye(128, dtype=np.float32)
    rcp = 1.0 / lensP.reshape(NCORES, 2, 128).astype(np.float32)  # [NC,2,128]
    diags = (eye[None, None] * rcp[:, :, :, None]).transpose(0, 2, 1, 3).reshape(
        NCORES, 128, 256)
    ident = np.broadcast_to(eye[None], (NCORES, 128, 128))
    c16 = np.concatenate(
        [diags, ident,
         np.broadcast_to(wexp[None, 0:128], (NCORES, 128, NE)),
         np.broadcast_to(wexp[None, 128:256], (NCORES, 128, NE))],
        axis=2).astype(np.float16)
    c1 = np.concatenate(
        [np.ones((1, 256), dtype=np.float32), be.reshape(1, NE)],
        axis=1).astype(np.float16)

    # static context rows in featT layout, permP order
    blocks = []
    for which in range(NB_CTX):
        blk = emb16[:, :, CTX_IDX[which], :]
        blkP = np.take_along_axis(blk, permP[:, :, None], 1)
        arr = blkP.reshape(NCORES, 2, 128, HC, 128).transpose(0, 4, 3, 1, 2)
        blocks.append(arr.reshape(NCORES, 128, HC * 256))
    ctxT = np.ascontiguousarray(np.concatenate(blocks, axis=2))

    # io36 + categories-as-float + 0/1 span masks (g0 then g1) + b_base
    MOFF = NE + 2
    cst32 = np.zeros((NCORES, 128, MOFF + JB + JA + 2), dtype=np.float32)
    cst32[:, :, :NE] = np.repeat(np.arange(NB_EXPERTS, dtype=np.float32),
                                 NB_LABELS)[None, None, :]
    cst32[:, :, NE:NE + 2] = catP.reshape(NCORES, 2, 128).transpose(0, 2, 1)
    for g, J in ((0, JB), (1, JA)):
        lens_g = lensP[:, g * 128:(g + 1) * 128].astype(np.float32)
        j = np.arange(J, dtype=np.float32)
        m = (j[None, None, :] < lens_g[:, :, None]).astype(np.float32)
        off = MOFF + g * JB
        cst32[:, :, off:off + J] = m
    cst32[:, :, MOFF + JB + JA:] = bbias.reshape(2, 128).T[None]

    in_maps = [
        {"embT": np.ascontiguousarray(emb16[i].reshape(EROWS, H)),
         "gidx": np.ascontiguousarray(gidx[i]),
         "wbT": wbT, "ctxT": ctxT[i],
         "c16": np.ascontiguousarray(c16[i]),
         "c32": np.ascontiguousarray(cst32[i]),
         "c1": np.ascontiguousarray(c1)}
        for i in range(NCORES)
    ]
    return {"in_maps": in_maps, "perm": permP, "key": (JA, JB)}


def _run(prep, **kw):
    nc = _get_nc(*prep["key"])
    return run_bass_kernel_spmd(nc, prep["in_maps"],
                                core_ids=list(range(NCORES)), **kw)


def _postprocess(prep, res):
    perm = prep["perm"]
    full = np.empty((B, NB_LABELS), dtype=np.float32)
    for i, r in enumerate(res.results):
        full[i * BC + perm[i]] = r["out"]
    return full


def kernel(embeddings, position_indexes, categories, W_base, b_base, W_experts,
           b_experts):
    prep = _prep_inputs(embeddings, position_indexes, categories, W_base,
                        b_base, W_experts, b_experts)
    res = _run(prep)
    return _postprocess(prep, res)
